# revision 1
# baseline (speedup 1.0000x reference)
"""InnerProductDecoder GNN edge-scoring kernel for 8 TRN2 NeuronCores.

Math: out[e] = (sigmoid(w * z[s]@(c@psi)[d]) + sigmoid(w * (c@psi)[s]@z[d])) / 2
Key identity: (c@psi)[s] . z[d] == c[s] . (z@psi.T)[d], so with zt = z@psi.T
both per-edge dots are K=64 dots against the packed table u = [c | zt] (N x 128
f32, 512B rows):
    v_cz[e] = u[s,0:64] . u[d,64:128]
    v_zc[e] = u[s,64:128] . u[d,0:64]

Per core: build the full u table in DRAM (PE matmul zt = z@psi.T), then for its
1/8 slice of edges dma_gather u[src], u[dst] (512B rows), DVE dot, ACT sigmoid.
dma_gather uses int16 indices, so the node table is split in two 25000-row
windows and edges are bucketed into 4 classes by (src-half, dst-half); each
class gathers with window-relative indices. Edge order is restored on host.
"""
import numpy as np

import concourse.bass as bass
import concourse.tile as tile
from concourse import bacc, mybir
from concourse.bass_utils import run_bass_kernel_spmd

N, D, K, E = 50000, 128, 64, 600000
NCORES = 8
HALF = N // 2          # int16 gather window size
EPC = E // NCORES      # edges per core
G = 2048               # edges per gather chunk
F32 = mybir.dt.float32
I16 = mybir.dt.int16


def _pack_idx(arr: np.ndarray) -> np.ndarray:
    """Gather-index layout: idx i -> partition i%16, col i//16; replicated 8x."""
    n = arr.shape[0]
    t = arr.astype(np.int16).reshape(n // 16, 16).T
    return np.tile(t, (8, 1))


def _build_bass(C: list[int]):
    """C[c] = padded per-class edge count (multiple of 128, same on all cores)."""
    TOT = sum(C)
    TOT16, TOTJ = TOT // 16, TOT // 128

    nc = bacc.Bacc("TRN2", target_bir_lowering=False, debug=False,
                   num_devices=NCORES)
    zt_in = nc.dram_tensor("zt", [D, N], F32, kind="ExternalInput")
    u_tab_t = nc.dram_tensor("utab", [N, D], F32, kind="ExternalInput")
    psit_in = nc.dram_tensor("psit", [D, K], F32, kind="ExternalInput")
    w_in = nc.dram_tensor("w", [1, 1], F32, kind="ExternalInput")
    s_in = nc.dram_tensor("sidx", [128, TOT16], I16, kind="ExternalInput")
    d_in = nc.dram_tensor("didx", [128, TOT16], I16, kind="ExternalInput")
    out = nc.dram_tensor("out", [128, TOTJ], F32, kind="ExternalOutput")

    with tile.TileContext(nc) as tc:
        with (
            tc.tile_pool(name="const", bufs=1) as cpool,
            tc.tile_pool(name="blda", bufs=3) as apool,
            tc.tile_pool(name="bldu", bufs=3) as upool,
            tc.tile_pool(name="psum", bufs=3, space="PSUM") as ppool,
            tc.tile_pool(name="gat", bufs=3) as gpool,
            tc.tile_pool(name="mul", bufs=3) as mpool,
            tc.tile_pool(name="red", bufs=3) as rpool,
        ):
            # --- constants ---
            psit_t = cpool.tile([D, K], F32)
            nc.sync.dma_start(psit_t[:], psit_in.ap())
            w_t = cpool.tile([1, 1], F32)
            nc.sync.dma_start(w_t[:], w_in.ap())
            w_b = cpool.tile([128, 1], F32)
            nc.gpsimd.partition_broadcast(w_b[:], w_t[:])
            sidx_t = cpool.tile([128, TOT16], I16)
            nc.sync.dma_start(sidx_t[:], s_in.ap())
            didx_t = cpool.tile([128, TOT16], I16)
            nc.sync.dma_start(didx_t[:], d_in.ap())
            out_sb = cpool.tile([128, TOTJ], F32)

            # --- phase A: write zt half into pre-filled u table ---
            u_tab = u_tab_t.ap()
            GR = 1024  # rows per build group
            r0 = 0
            while r0 < N:
                rows = min(GR, N - r0)
                nfull = rows // 128          # full 128-row sub-tiles
                rem = rows - nfull * 128     # tail rows (< 128)
                zt_blk = apool.tile([128, GR], F32, tag="zt")
                nc.sync.dma_start(zt_blk[:, :rows], zt_in.ap()[:, r0:r0 + rows])
                u_big = upool.tile([128, GR // 128, K], F32, tag="ub")
                ps = ppool.tile([128, 512], F32, tag="ps")
                nsub = nfull + (1 if rem else 0)
                for g in range(nsub):
                    sr = 128 if g < nfull else rem
                    nc.tensor.matmul(
                        out=ps[:sr, g * K:(g + 1) * K],
                        lhsT=zt_blk[:, g * 128:g * 128 + sr],
                        rhs=psit_t[:],
                        start=True, stop=True,
                    )
                # copy zt into SBUF, then write zt half-rows (bytes 256:512)
                if nfull:
                    nc.vector.tensor_copy(
                        u_big[:, :nfull, :],
                        ps[:, 0:nfull * K].rearrange("p (g k) -> p g k", k=K),
                    )
                    nc.sync.dma_start(
                        u_tab[r0:r0 + nfull * 128, K:D].rearrange(
                            "(g p) d -> p g d", p=128),
                        u_big[:, :nfull, :],
                    )
                if rem:
                    nc.vector.tensor_copy(
                        u_big[:rem, nfull, :], ps[:rem, nfull * K:(nfull + 1) * K])
                    nc.sync.dma_start(
                        u_tab[r0 + nfull * 128:r0 + rows, K:D],
                        u_big[:rem, nfull, :],
                    )
                r0 += rows

            # --- phase B: gather + dot + sigmoid ---
            col = 0   # running offset (in edges) into idx/out arrays
            for cls in range(4):
                ws, wd = cls >> 1, cls & 1
                u_s = u_tab[ws * HALF:(ws + 1) * HALF, :]
                u_d = u_tab[wd * HALF:(wd + 1) * HALF, :]
                done = 0
                while done < C[cls]:
                    g = min(G, C[cls] - done)
                    j = g // 128
                    c16, cj = col // 16, col // 128
                    s_t = gpool.tile([128, G // 128, D], F32, tag="st")
                    nc.gpsimd.dma_gather(
                        s_t[:, :j, :], u_s, sidx_t[:, c16:c16 + g // 16],
                        num_idxs=g, num_idxs_reg=g, elem_size=D, single_packet=False)
                    d_t = gpool.tile([128, G // 128, D], F32, tag="dt")
                    nc.gpsimd.dma_gather(
                        d_t[:, :j, :], u_d, didx_t[:, c16:c16 + g // 16],
                        num_idxs=g, num_idxs_reg=g, elem_size=D, single_packet=False)
                    m1 = mpool.tile([128, G // 128, K], F32, tag="m1")
                    nc.vector.tensor_tensor(
                        out=m1[:, :j, :], in0=s_t[:, :j, 0:K], in1=d_t[:, :j, K:D],
                        op=mybir.AluOpType.mult)
                    m2 = mpool.tile([128, G // 128, K], F32, tag="m2")
                    nc.vector.tensor_tensor(
                        out=m2[:, :j, :], in0=s_t[:, :j, K:D], in1=d_t[:, :j, 0:K],
                        op=mybir.AluOpType.mult)
                    r1 = rpool.tile([128, G // 128], F32, tag="r1")
                    nc.vector.tensor_reduce(
                        out=r1[:, :j], in_=m1[:, :j, :], axis=mybir.AxisListType.X,
                        op=mybir.AluOpType.add)
                    r2 = rpool.tile([128, G // 128], F32, tag="r2")
                    nc.vector.tensor_reduce(
                        out=r2[:, :j], in_=m2[:, :j, :], axis=mybir.AxisListType.X,
                        op=mybir.AluOpType.add)
                    sg1 = rpool.tile([128, G // 128], F32, tag="sg1")
                    nc.scalar.activation(
                        sg1[:, :j], r1[:, :j], mybir.ActivationFunctionType.Sigmoid,
                        scale=w_b[:])
                    sg2 = rpool.tile([128, G // 128], F32, tag="sg2")
                    nc.scalar.activation(
                        sg2[:, :j], r2[:, :j], mybir.ActivationFunctionType.Sigmoid,
                        scale=w_b[:])
                    sm = rpool.tile([128, G // 128], F32, tag="sm")
                    nc.vector.tensor_tensor(
                        out=sm[:, :j], in0=sg1[:, :j], in1=sg2[:, :j],
                        op=mybir.AluOpType.add)
                    nc.vector.tensor_scalar(
                        out=out_sb[:, cj:cj + j], in0=sm[:, :j],
                        scalar1=0.5, scalar2=None, op0=mybir.AluOpType.mult)
                    done += g
                    col += g

            nc.sync.dma_start(out.ap(), out_sb[:])
    nc.compile()
    return nc


def prepare(z, c, psi, weights, edge_index):
    z = np.asarray(z, dtype=np.float32)
    c = np.asarray(c, dtype=np.float32)
    psi = np.asarray(psi, dtype=np.float32)
    weights = np.asarray(weights, dtype=np.float32)
    ei = np.asarray(edge_index).astype(np.int64)

    zt = np.ascontiguousarray(z.T)                    # [D, N]
    psit = np.ascontiguousarray(psi.T)                # [D, K]
    w = weights.reshape(1, 1)
    utab_init = np.zeros((N, D), dtype=np.float32)
    utab_init[:, 0:K] = c

    # --- host: bucket each core's edges into 4 (src-half, dst-half) classes ---
    src_all = ei[0].astype(np.int32)
    dst_all = ei[1].astype(np.int32)
    per_core = []
    counts = np.zeros((NCORES, 4), dtype=np.int64)
    for i in range(NCORES):
        s = src_all[i * EPC:(i + 1) * EPC]
        d = dst_all[i * EPC:(i + 1) * EPC]
        cls = ((s >= HALF).astype(np.int32) << 1) | (d >= HALF).astype(np.int32)
        order = np.argsort(cls, kind="stable")
        per_core.append((s, d, cls, order))
        for cc in range(4):
            counts[i, cc] = int((cls == cc).sum())
    C = [int(-(-counts[:, cc].max() // 128) * 128) for cc in range(4)]

    nc = _build_bass(C)

    in_maps = []
    for i in range(NCORES):
        s, d, cls, order = per_core[i]
        s_sorted, d_sorted, cls_sorted = s[order], d[order], cls[order]
        s_seg, d_seg = [], []
        base = 0
        for cc in range(4):
            n = int(counts[i, cc])
            pad = C[cc] - n
            ws, wd = cc >> 1, cc & 1
            s_rel = np.concatenate([s_sorted[base:base + n] - ws * HALF,
                                    np.zeros(pad, dtype=np.int32)])
            d_rel = np.concatenate([d_sorted[base:base + n] - wd * HALF,
                                    np.zeros(pad, dtype=np.int32)])
            s_seg.append(_pack_idx(s_rel))
            d_seg.append(_pack_idx(d_rel))
            base += n
        in_maps.append({
            "zt": zt, "utab": utab_init, "psit": psit, "w": w,
            "sidx": np.ascontiguousarray(np.concatenate(s_seg, axis=1)),
            "didx": np.ascontiguousarray(np.concatenate(d_seg, axis=1)),
        })

    return nc, in_maps, (per_core, counts, C)


def unshard(results, meta):
    per_core, counts, C = meta
    final = np.empty(E, dtype=np.float32)
    offj = np.cumsum([0] + [cc // 128 for cc in C])
    for i in range(NCORES):
        s, d, cls, order = per_core[i]
        dev = results[i]["out"]                # [128, TOTJ]
        base = 0
        for cc in range(4):
            n = int(counts[i, cc])
            blk = dev[:, offj[cc]:offj[cc + 1]]          # [128, C[cc]//128]
            vals = blk.T.ravel()[:n]
            final[i * EPC + order[base:base + n]] = vals
            base += n
    return final


def kernel(z, c, psi, weights, edge_index):
    nc, in_maps, meta = prepare(z, c, psi, weights, edge_index)
    res = run_bass_kernel_spmd(nc, in_maps, core_ids=list(range(NCORES)))
    kernel.last_results = res
    return unshard(res.results, meta)

